# revision 2
# baseline (speedup 1.0000x reference)
"""RLeaky SNN scan kernel v3 for Trainium2 (8 NeuronCores, batch data-parallel).

Structure as v2 (W-moving matmul, 4-way col-group tiling, XBAR spike
transposes), with two accuracy upgrades that bring the whole scan to within
~1-2 fp32 ulp per step of the fp32 reference:

1. fp16 hi/lo weight split, x256 scaled: W.T*256 = hi + lo with hi,lo fp16.
   Residual ~2^-24 relative (fp32-ulp level) since fp16 carries 11 mantissa
   bits per part. Spikes are exactly 0/1, so every matmul product is exact;
   PSUM accumulates 256*dot whose fp32 ripple rounding is bitwise-equivalent
   to the unscaled sum (pure exponent shift). A fused DVE op descales by
   2^-8 exactly.

2. The elementwise chain replicates the XLA reference's association order:
   u2 = (0.95*mem) + x_t ; u3 = u2 + dot ; u4 = u3 + b ; mem' = u4 - spk ;
   spk' = (mem' > 1).

Layout L2: tile [128, 512]; partition p = 32*g + i (strip g, batch i),
free col c; element (batch i, f = 512*g + c).
"""

import sys

if "/opt/trn_rl_repo" not in sys.path:
    sys.path.insert(0, "/opt/trn_rl_repo")

import numpy as np

import concourse.mybir as mybir
import concourse.tile as tile
from concourse import bacc
from concourse.bass_utils import run_bass_kernel_spmd

F32 = mybir.dt.float32
F16 = mybir.dt.float16

B, T_FULL, F = 256, 128, 2048
NCORES = 8
BL = B // NCORES  # 32 batch rows per core
NG = 4
NJB = 16
WSCALE = 256.0

# half-0 covers f-chunks {4g, 4g+1}, half-1 covers {4g+2, 4g+3}
CHUNK_ORDER = [0, 1, 4, 5, 8, 9, 12, 13, 2, 3, 6, 7, 10, 11, 14, 15]

_nc_cache = {}


def _build(T=T_FULL):
    if T in _nc_cache:
        return _nc_cache[T]

    nc = bacc.Bacc(None, target_bir_lowering=False)
    x_d = nc.dram_tensor("x", [T, 128, 512], F32, kind="ExternalInput")
    # hi/lo fp16 split of 256*W.T
    wt_d = nc.dram_tensor("wt", [2, NJB, 128, F], F16, kind="ExternalInput")
    b_d = nc.dram_tensor("b", [128, 512], F32, kind="ExternalInput")
    spk_out = nc.dram_tensor("spk_out", [T, 128, 512], F16, kind="ExternalOutput")
    mem_out = nc.dram_tensor("mem_out", [T, 128, 512], F32, kind="ExternalOutput")

    with tile.TileContext(nc) as tc:
        with (
            tc.tile_pool(name="wpool", bufs=1) as wpool,
            tc.tile_pool(name="const", bufs=1) as const,
            tc.tile_pool(name="xtp", bufs=3) as xtp,
            tc.tile_pool(name="state", bufs=2) as state,
            tc.tile_pool(name="spkp", bufs=2) as spkp,
            tc.tile_pool(name="spktp", bufs=2) as spktp,
            tc.tile_pool(name="tmp", bufs=2) as tmp,
            tc.tile_pool(name="pmm", bufs=4, space="PSUM") as pmm,
        ):
            # --- init: W.T hi/lo chunks resident in SBUF (fp16, 16MB)
            w_sb = [[], []]
            for p in range(2):
                for jb in range(NJB):
                    wc = wpool.tile([128, F], F16, tag=f"w{p}_{jb}", name=f"w{p}_{jb}")
                    nc.scalar.dma_start(wc[:], wt_d[p, jb, :, :])
                    w_sb[p].append(wc)

            b_sb = const.tile([128, 512], F32)
            nc.scalar.dma_start(b_sb[:], b_d[:])

            mem_cur = state.tile([128, 512], F32, tag="mem", name="mem0")
            nc.vector.memset(mem_cur[:], 0.0)
            spk_cur = [None, None]
            for h in range(2):
                s = spkp.tile([128, 256], F16, tag=f"spk{h}", name=f"spk{h}_0")
                nc.vector.memset(s[:], 0.0)
                spk_cur[h] = s
            spkT_cur = spktp.tile([128, NJB, 32], F16, tag="spkT", name="spkT0")
            nc.vector.memset(spkT_cur[:], 0.0)

            for t in range(T):
                xt = xtp.tile([128, 512], F32, tag="xt", name=f"xt{t}")
                nc.scalar.dma_start(xt[:], x_d[t, :, :])

                # --- PE: dot*256 accumulated over chunks; halves pipelined
                ps = [None, None]
                for h in range(2):
                    ps_h = pmm.tile([128, 256], F32, tag=f"ps{h}", name=f"ps{h}_{t}")
                    for k, jb in enumerate(CHUNK_ORDER):
                        for p in range(2):
                            for g in range(NG):
                                nc.tensor.matmul(
                                    ps_h[32 * g : 32 * (g + 1), :],
                                    spkT_cur[:, jb, :],
                                    w_sb[p][jb][
                                        :, 512 * g + 256 * h : 512 * g + 256 * h + 256
                                    ],
                                    start=(k == 0 and p == 0),
                                    stop=(k == NJB - 1 and p == 1),
                                    tile_position=(0, 32 * g),
                                )
                    ps[h] = ps_h

                # --- DVE chain in the reference's association order
                u2 = tmp.tile([128, 512], F32, tag="u2", name=f"u2_{t}")
                for h in range(2):
                    ch = slice(256 * h, 256 * (h + 1))
                    # u2 = (0.95 * mem) + x
                    nc.vector.scalar_tensor_tensor(
                        u2[:, ch], mem_cur[:, ch], 0.95, xt[:, ch],
                        mybir.AluOpType.mult, mybir.AluOpType.add,
                    )

                mem_new = state.tile([128, 512], F32, tag="mem", name=f"mem{t + 1}")
                u3 = tmp.tile([128, 512], F32, tag="u3", name=f"u3_{t}")
                u4 = tmp.tile([128, 512], F32, tag="u4", name=f"u4_{t}")
                spk_new = [None, None]
                spkT_new = None
                if t + 1 < T:
                    spkT_new = spktp.tile(
                        [128, NJB, 32], F16, tag="spkT", name=f"spkT{t + 1}"
                    )
                for h in range(2):
                    ch = slice(256 * h, 256 * (h + 1))
                    # u3 = u2 + dot  (descale PSUM by 2^-8 exactly, then add)
                    nc.vector.scalar_tensor_tensor(
                        u3[:, ch], ps[h][:], 1.0 / WSCALE, u2[:, ch],
                        mybir.AluOpType.mult, mybir.AluOpType.add,
                    )
                    # u4 = u3 + b
                    nc.vector.tensor_add(u4[:, ch], u3[:, ch], b_sb[:, ch])
                    # mem' = u4 - spk
                    nc.vector.tensor_sub(mem_new[:, ch], u4[:, ch], spk_cur[h][:])
                    s = spkp.tile(
                        [128, 256], F16, tag=f"spk{h}", name=f"spk{h}_{t + 1}"
                    )
                    nc.vector.tensor_scalar(
                        s[:], mem_new[:, ch], 1.0, None, mybir.AluOpType.is_gt
                    )
                    spk_new[h] = s
                    if t + 1 < T:
                        for g in range(NG):
                            nc.sync.dma_start_transpose(
                                spkT_new[:, 4 * g + 2 * h : 4 * g + 2 * h + 2, :],
                                s[32 * g : 32 * (g + 1), :],
                            )
                    nc.gpsimd.dma_start(spk_out[t, :, ch], s[:])

                nc.gpsimd.dma_start(mem_out[t, :, :], mem_new[:])

                mem_cur = mem_new
                spk_cur = spk_new
                if spkT_new is not None:
                    spkT_cur = spkT_new

    nc.compile()
    _nc_cache[T] = nc
    return nc


def _pack_l2(xc, T):
    # [32, T, 2048] -> [T, 128, 512] L2: out[t, 32g+i, c] = xc[i, t, 512g+c]
    a = xc.transpose(1, 0, 2).reshape(T, BL, NG, 512).transpose(0, 2, 1, 3)
    return np.ascontiguousarray(a.reshape(T, 128, 512), dtype=np.float32)


def _unpack_rec(a, T):
    # [T, 128, 512] L2 -> [32, T, 2048]
    a = np.asarray(a, dtype=np.float32).reshape(T, NG, BL, 512)
    a = a.transpose(2, 0, 1, 3).reshape(BL, T, F)
    return np.ascontiguousarray(a)


def make_in_maps(x, W, b, T):
    ws = (W.T.astype(np.float32) * WSCALE).astype(np.float32)
    wt_hi = ws.astype(np.float16)
    wt_lo = (ws - wt_hi.astype(np.float32)).astype(np.float16)
    wt = np.ascontiguousarray(np.stack([wt_hi, wt_lo]).reshape(2, NJB, 128, F))
    # b_l2[32g+i, c] = b[512g+c]
    b_l2 = np.ascontiguousarray(
        np.broadcast_to(
            b.reshape(NG, 1, 512), (NG, BL, 512)
        ).reshape(128, 512),
        dtype=np.float32,
    )
    in_maps = []
    for c in range(NCORES):
        xc = x[c * BL : (c + 1) * BL]  # [32, T, 2048]
        in_maps.append({"x": _pack_l2(xc, T), "wt": wt, "b": b_l2})
    return in_maps


def unpack_results(results, T):
    spk_parts = []
    mem_parts = []
    for c in range(NCORES):
        spk_parts.append(_unpack_rec(results[c]["spk_out"], T))
        mem_parts.append(_unpack_rec(results[c]["mem_out"], T))
    return np.concatenate(spk_parts, axis=0), np.concatenate(mem_parts, axis=0)


def kernel(x, W, b, T=None, trace=False):
    x = np.asarray(x, dtype=np.float32)
    W = np.asarray(W, dtype=np.float32)
    b = np.asarray(b, dtype=np.float32)
    if T is None:
        T = x.shape[1]
    x = x[:, :T, :]

    nc = _build(T)
    in_maps = make_in_maps(x, W, b, T)

    try:
        res = run_bass_kernel_spmd(
            nc, in_maps, core_ids=list(range(NCORES)), trace=trace
        )
    except ModuleNotFoundError:
        res = run_bass_kernel_spmd(
            nc, in_maps, core_ids=list(range(NCORES)), trace=False
        )
    spk_rec, mem_rec = unpack_results(res.results, T)
    if trace:
        kernel.last_result = res
    return spk_rec, mem_rec
